# revision 11
# baseline (speedup 1.0000x reference)
"""Trainium2 Bass kernel for nn_Attention (Bahdanau attention + LSTM decoder scan).

Data-parallel over batch B=512 across 8 NeuronCores (64 rows/core, no
collectives).  Per core:
  hoist:  pHT[h,(t,b)] = i2h_w @ batch_H^T   (PE, written to DRAM, bf16)
  scan (26 steps):
    phT   = h2h_w @ h^T                       (PE)
    z     = pHT + broadcast(phT); T = tanh(z) (DVE 2x + ACT), streamed from DRAM
    acc   = sum_c w_c * T_c                   (DVE scalar_tensor_tensor chain)
    e^T   = ones^T-reduce over 128 h-rem      (PE, acc blocks as weights)
    alpha = softmax_t(e)                      (PE transposes + ACT exp + DVE)
    ctx^T = bHC-tiles(weights) @ alpha_b      (PE, per-b matvec)
    gates = [ctx;h]^T-chunks @ Wg + tok_bias  (PE, Wg streamed from DRAM)
    LSTM pointwise (sigmoid via tanh table)   (ACT+DVE)
    logits^T = gen^T-tiles @ h^T              (PE) -> DRAM out
All matmul operands bf16 (f32 PSUM accum); c-state f32.
"""
import sys
from contextlib import ExitStack

import numpy as np
import ml_dtypes

sys.path.insert(0, "/opt/trn_rl_repo")

import concourse.bass as bass  # noqa: E402
from concourse import bacc  # noqa: E402
import concourse.tile as tile  # noqa: E402
from concourse import mybir  # noqa: E402
from concourse.bass_utils import run_bass_kernel_spmd  # noqa: E402

BF16 = ml_dtypes.bfloat16
LAST_RESULT = None
BF = mybir.dt.bfloat16
F32 = mybir.dt.float32
ADD = mybir.AluOpType.add
MULT = mybir.AluOpType.mult
TANH = mybir.ActivationFunctionType.Tanh
EXP = mybir.ActivationFunctionType.Exp

B, T, D, H, V, STEPS = 512, 128, 1024, 1024, 256, 26
NC_ = 8
BL = B // NC_          # 64 batch rows per core
TB = T * BL            # 8192 (t,b) columns, t outer / b inner
FB = 2048              # free-block of (t,b) columns
NF = TB // FB          # 4
TFB = FB // BL         # 32 t-values per free block
HC = 8                 # 128-chunks of H
DC = 8                 # 128-chunks of D
KG = 16                # gates K chunks (ctx 8 + h 8)
G4 = 4 * H             # 4096


def build_kernel():
    nc = bacc.Bacc("TRN2", target_bir_lowering=False, debug=False)
    # inputs
    bHT = nc.declare_dram_parameter("bHT", [D, TB], BF, isOutput=False)
    bHC = nc.declare_dram_parameter("bHC", [T, BL * D], BF, isOutput=False)
    i2hT = nc.declare_dram_parameter("i2hT", [D, H], BF, isOutput=False)
    h2hT = nc.declare_dram_parameter("h2hT", [H, H], BF, isOutput=False)
    wsc = nc.declare_dram_parameter("wsc", [128, HC], F32, isOutput=False)
    h2hbT = nc.declare_dram_parameter("h2hbT", [128, HC], F32, isOutput=False)
    WgT = nc.declare_dram_parameter("WgT", [2 * H, G4], BF, isOutput=False)
    tb = nc.declare_dram_parameter("tb", [STEPS, BL, G4], BF, isOutput=False)
    genT = nc.declare_dram_parameter("genT", [H, V], BF, isOutput=False)
    genbT = nc.declare_dram_parameter("genbT", [128, 2], F32, isOutput=False)
    ident = nc.declare_dram_parameter("ident", [128, 128], BF, isOutput=False)
    onesc = nc.declare_dram_parameter("onesc", [128, 1], BF, isOutput=False)
    out = nc.declare_dram_parameter("out", [STEPS, 2, 128, BL], F32, isOutput=True)
    # internal scratch in DRAM
    pHT = nc.dram_tensor("pHT", [HC, 128, TB], BF)

    ctx = ExitStack()
    tc = ctx.enter_context(tile.TileContext(nc))

    # ---------------- persistent tiles ----------------
    res = ctx.enter_context(tc.tile_pool(name="res", bufs=1))
    bHC_sb = res.tile([T, BL * D], BF, name="bHC_sb")        # 128 KiB/part
    h2hT_sb = res.tile([128, HC * H], BF, name="h2hT_sb")    # 16 KiB/part
    wsc_sb = res.tile([128, HC], F32, name="wsc_sb")
    h2hbT_sb = res.tile([128, HC], F32, name="h2hbT_sb")
    genbT_sb = res.tile([128, 2], F32, name="genbT_sb")
    ident_sb = res.tile([128, 128], BF, name="ident_sb")
    ones_sb = res.tile([128, 1], BF, name="ones_sb")
    hT_sb = res.tile([128, HC * BL], BF, name="hT_sb")       # h^T chunks [h,b]
    cB_sb = res.tile([BL, H], F32, name="cB_sb")             # c state (b-layout)
    phT_sb = res.tile([128, HC * BL], BF, name="phT_sb")
    ctxT_sb = res.tile([128, DC * BL], BF, name="ctxT_sb")
    gates_sb = res.tile([BL, G4], BF, name="gates_sb")
    genT_sb = res.tile([128, HC * V], BF, name="genT_sb")

    nc.sync.dma_start(bHC_sb[:], bHC[:])
    for k in range(HC):
        nc.sync.dma_start(h2hT_sb[:, k * H:(k + 1) * H], h2hT[k * 128:(k + 1) * 128, :])
    nc.sync.dma_start(wsc_sb[:], wsc[:])
    nc.sync.dma_start(h2hbT_sb[:], h2hbT[:])
    nc.sync.dma_start(genbT_sb[:], genbT[:])
    nc.sync.dma_start(ident_sb[:], ident[:])
    nc.sync.dma_start(ones_sb[:], onesc[:])
    for k in range(HC):
        nc.sync.dma_start(genT_sb[:, k * V:(k + 1) * V],
                          genT[k * 128:(k + 1) * 128, :])
    nc.vector.memset(hT_sb[:], 0.0)
    nc.vector.memset(cB_sb[:], 0.0)

    # ---------------- hoist: pHT = i2h @ bH^T ----------------
    with tc.tile_pool(name="hoist", bufs=1) as hres, \
         tc.tile_pool(name="hrhs", bufs=2) as hrhs, \
         tc.tile_pool(name="hst", bufs=3) as hst, \
         tc.tile_pool(name="hps", bufs=4, space="PSUM") as hps:
        i2hT_sb = hres.tile([128, HC * H], BF, name="i2hT_sb")
        for k in range(DC):
            nc.sync.dma_start(i2hT_sb[:, k * H:(k + 1) * H],
                              i2hT[k * 128:(k + 1) * 128, :])
        for n in range(TB // 512):
            rt = hrhs.tile([128, DC * 512], BF, name="hoist_rhs", tag="hrhs")
            for k in range(DC):
                nc.sync.dma_start(rt[:, k * 512:(k + 1) * 512],
                                  bHT[k * 128:(k + 1) * 128, n * 512:(n + 1) * 512])
            for c in range(HC):
                ps = hps.tile([128, 512], F32, name="hoist_ps")
                for k in range(DC):
                    nc.tensor.matmul(ps[:], i2hT_sb[:, k * H + c * 128: k * H + (c + 1) * 128],
                                     rt[:, k * 512:(k + 1) * 512],
                                     start=(k == 0), stop=(k == DC - 1))
                st = hst.tile([128, 512], BF, name="hoist_stage")
                nc.vector.tensor_copy(st[:], ps[:])
                nc.gpsimd.dma_start(pHT[c, :, n * 512:(n + 1) * 512], st[:])

    # ---------------- scan pools ----------------
    phs = ctx.enter_context(tc.tile_pool(name="phs", bufs=2))       # pH stream
    zp = ctx.enter_context(tc.tile_pool(name="zp", bufs=2))         # z tiles
    accp = ctx.enter_context(tc.tile_pool(name="accp", bufs=1))     # chain acc
    wgp = ctx.enter_context(tc.tile_pool(name="wgp", bufs=4))       # Wg stream
    tbp = ctx.enter_context(tc.tile_pool(name="tbp", bufs=2))       # tok bias
    smx = ctx.enter_context(tc.tile_pool(name="smx", bufs=1))       # softmax bits
    lst = ctx.enter_context(tc.tile_pool(name="lst", bufs=1))       # LSTM tmp
    lout = ctx.enter_context(tc.tile_pool(name="lout", bufs=1))     # logits stage

    ph_ps = ctx.enter_context(tc.tile_pool(name="ph_ps", bufs=1, space="PSUM"))
    e_ps = ctx.enter_context(tc.tile_pool(name="e_ps", bufs=1, space="PSUM"))
    tr_ps = ctx.enter_context(tc.tile_pool(name="tr_ps", bufs=1, space="PSUM"))
    cx_ps = ctx.enter_context(tc.tile_pool(name="cx_ps", bufs=1, space="PSUM"))
    g_ps = ctx.enter_context(tc.tile_pool(name="g_ps", bufs=1, space="PSUM"))
    ht_ps = ctx.enter_context(tc.tile_pool(name="ht_ps", bufs=1, space="PSUM"))
    lo_ps = ctx.enter_context(tc.tile_pool(name="lo_ps", bufs=1, space="PSUM"))

    for s in range(STEPS):
        # ---- 1. phT = h2h @ h^T + bias ----
        for c in range(HC):
            pp = ph_ps.tile([128, BL], F32, name="ph_ps_t")
            for k in range(HC):
                nc.tensor.matmul(pp[:], h2hT_sb[:, k * H + c * 128: k * H + (c + 1) * 128],
                                 hT_sb[:, k * BL:(k + 1) * BL],
                                 start=(k == 0), stop=(k == HC - 1))
            nc.vector.tensor_scalar(phT_sb[:, c * BL:(c + 1) * BL], pp[:],
                                    h2hbT_sb[:, c:c + 1], None, ADD)
        # ---- 2. z/tanh/chain + per-f score reduce ----
        ep = e_ps.tile([128, BL], F32, name="eT_ps")
        for f in range(NF):
            acc = accp.tile([128, FB], BF, name="acc_t")
            for c in range(HC):
                pht = phs.tile([128, FB], BF, name="pH_t")
                nc.sync.dma_start(pht[:], pHT[c, :, f * FB:(f + 1) * FB])
                z = zp.tile([128, FB], BF, name="z_t")
                z3 = z[:].rearrange("p (t b) -> p t b", b=BL)
                ph3 = pht[:].rearrange("p (t b) -> p t b", b=BL)
                phb = phT_sb[:, c * BL:(c + 1) * BL].unsqueeze(1).broadcast_to(
                    (128, TFB, BL))
                nc.vector.tensor_tensor(z3, ph3, phb, ADD)
                nc.scalar.activation(z[:], z[:], TANH)
                if c == 0:
                    nc.vector.tensor_scalar(acc[:], z[:], wsc_sb[:, 0:1], None, MULT)
                else:
                    nc.vector.scalar_tensor_tensor(acc[:], z[:], wsc_sb[:, c:c + 1],
                                                   acc[:], MULT, ADD)
            # score reduce rows t in [f*TFB, (f+1)*TFB): strided lhsT per b
            acc3 = acc[:].rearrange("p (t b) -> p b t", b=BL)  # [128, 64, 32]
            for bb in range(BL):
                nc.tensor.matmul(ep[f * TFB:(f + 1) * TFB, bb:bb + 1],
                                 acc3[:, bb, :], ones_sb[:],
                                 start=True, stop=True,
                                 tile_position=(0, f * TFB))
        # ---- 4. softmax over t ----
        e1 = smx.tile([128, BL], BF, name="e1")
        nc.vector.tensor_copy(e1[:], ep[:])
        tp = tr_ps.tile([128, 128], BF, name="tr_t", tag="tr")[0:BL, :]
        nc.tensor.transpose(tp[:], e1[:], ident_sb[:])
        e2 = smx.tile([BL, 128], BF, name="e2")
        nc.vector.tensor_copy(e2[:], tp[:])
        ex = smx.tile([BL, 128], F32, name="ex")
        nc.scalar.activation(ex[:], e2[:], EXP)
        sg = smx.tile([BL, 1], F32, name="sig")
        nc.vector.tensor_reduce(sg[:], ex[:], mybir.AxisListType.X, ADD)
        rc = smx.tile([BL, 1], F32, name="rec")
        nc.vector.reciprocal(rc[:], sg[:])
        al = smx.tile([BL, 128], BF, name="alpha")
        nc.vector.tensor_scalar(al[:], ex[:], rc[:], None, MULT)
        ap2 = tr_ps.tile([128, 128], BF, name="tr_t2", tag="tr")[:, 0:BL]
        nc.tensor.transpose(ap2[:], al[:], ident_sb[0:BL, 0:BL])
        alT = smx.tile([128, BL], BF, name="alT")
        nc.vector.tensor_copy(alT[:], ap2[:])
        # ---- 5. ctx^T[d, b] ----
        cxp = cx_ps.tile([128, DC * BL], F32, name="ctxT_ps")
        for bb in range(BL):
            for c in range(DC):
                nc.tensor.matmul(cxp[:, c * BL + bb: c * BL + bb + 1],
                                 bHC_sb[:, bb * D + c * 128: bb * D + (c + 1) * 128],
                                 alT[:, bb:bb + 1], start=True, stop=True)
        nc.vector.tensor_copy(ctxT_sb[:], cxp[:])
        # ---- 6. gates = [ctx; h]^T @ Wg + tb ----
        for nb in range(G4 // 512):
            gp = g_ps.tile([BL, 512], F32, name="g_ps_t")
            for k in range(KG):
                wg = wgp.tile([128, 512], BF, name="wg_t")
                nc.sync.dma_start(wg[:], WgT[k * 128:(k + 1) * 128, nb * 512:(nb + 1) * 512])
                xk = (ctxT_sb[:, k * BL:(k + 1) * BL] if k < DC
                      else hT_sb[:, (k - DC) * BL:(k - DC + 1) * BL])
                nc.tensor.matmul(gp[:], xk, wg[:], start=(k == 0), stop=(k == KG - 1))
            tbt = tbp.tile([BL, 512], BF, name="tb_t")
            nc.sync.dma_start(tbt[:], tb[s, :, nb * 512:(nb + 1) * 512])
            nc.vector.scalar_tensor_tensor(gates_sb[:, nb * 512:(nb + 1) * 512],
                                           gp[:], 1.0, tbt[:], MULT, ADD)
        # ---- 7. LSTM pointwise (sigmoid via tanh), in-place on gates ----
        sif = gates_sb[:, 0:2 * H]
        so = gates_sb[:, 3 * H:4 * H]
        tg = gates_sb[:, 2 * H:3 * H]
        nc.scalar.activation(sif, sif, TANH, scale=0.5)
        nc.scalar.activation(so, so, TANH, scale=0.5)
        nc.scalar.activation(tg, tg, TANH)
        nc.vector.tensor_scalar(sif, sif, 0.5, 0.5, MULT, ADD)
        nc.vector.tensor_scalar(so, so, 0.5, 0.5, MULT, ADD)
        m1 = lst.tile([BL, H], BF, name="m1")
        nc.vector.tensor_tensor(m1[:], gates_sb[:, 0:H], tg, MULT)
        nc.vector.tensor_tensor(cB_sb[:], gates_sb[:, H:2 * H], cB_sb[:], MULT)
        nc.vector.tensor_tensor(cB_sb[:], cB_sb[:], m1[:], ADD)
        th = lst.tile([BL, H], BF, name="th")
        nc.scalar.activation(th[:], cB_sb[:], TANH)
        hB = lst.tile([BL, H], BF, name="hB")
        nc.vector.tensor_tensor(hB[:], so, th[:], MULT)
        # ---- 8. transpose h -> hT ----
        for k in range(HC):
            hp = ht_ps.tile([128, BL], BF, name="hT_ps_t")
            nc.tensor.transpose(hp[:], hB[:, k * 128:(k + 1) * 128],
                                ident_sb[0:BL, 0:BL])
            nc.vector.tensor_copy(hT_sb[:, k * BL:(k + 1) * BL], hp[:])
        # ---- 9. logits^T ----
        for vc in range(2):
            lp = lo_ps.tile([128, BL], F32, name="lo_ps_t")
            for k in range(HC):
                nc.tensor.matmul(lp[:], genT_sb[:, k * V + vc * 128: k * V + (vc + 1) * 128],
                                 hT_sb[:, k * BL:(k + 1) * BL],
                                 start=(k == 0), stop=(k == HC - 1))
            lo = lout.tile([128, BL], F32, name="lo_st")
            nc.vector.tensor_scalar(lo[:], lp[:], genbT_sb[:, vc:vc + 1], None, ADD)
            nc.sync.dma_start(out[s, vc], lo[:])

    ctx.close()
    nc.compile()
    return nc


def _prep_core(ci, batch_H, text, i2h_w, h2h_w, h2h_b, score_w, W_ih, W_hh,
               b_ih, b_hh, gen_w, gen_b, shared):
    bH = batch_H[ci * BL:(ci + 1) * BL]          # [64, 128, 1024] f32
    tx = text[ci * BL:(ci + 1) * BL]             # [64, 26]
    # bHT [d, (t,b)]: bHT[d, t*64+b] = bH[b, t, d]
    bHT = np.ascontiguousarray(bH.transpose(2, 1, 0).reshape(D, TB)).astype(BF16)
    # bHC [t, b*D+d]
    bHC = np.ascontiguousarray(bH.transpose(1, 0, 2).reshape(T, BL * D)).astype(BF16)
    # tok bias [26, 64, 4096]
    tbv = shared["Wtok"][:, tx.astype(np.int64)].transpose(2, 1, 0)  # [26,64,4096]
    tbv = np.ascontiguousarray(tbv).astype(BF16)
    m = dict(shared["const"])
    m.update({"bHT": bHT, "bHC": bHC, "tb": tbv})
    return m


def kernel(batch_H, text, i2h_w, h2h_w, h2h_b, score_w, W_ih, W_hh, b_ih, b_hh,
           gen_w, gen_b):
    batch_H = np.asarray(batch_H, dtype=np.float32)
    text = np.asarray(text)
    f32 = lambda x: np.asarray(x, dtype=np.float32)
    i2h_w, h2h_w, h2h_b = f32(i2h_w), f32(h2h_w), f32(h2h_b)
    score_w, W_ih, W_hh = f32(score_w), f32(W_ih), f32(W_hh)
    b_ih, b_hh, gen_w, gen_b = f32(b_ih), f32(b_hh), f32(gen_w), f32(gen_b)

    Wtok = (W_ih[:, D:] + (b_ih + b_hh)[:, None]).astype(np.float32)  # [4096, 256]
    const = {
        "i2hT": np.ascontiguousarray(i2h_w.T).astype(BF16),
        "h2hT": np.ascontiguousarray(h2h_w.T).astype(BF16),
        "wsc": np.ascontiguousarray(score_w[0].reshape(HC, 128).T).astype(np.float32),
        "h2hbT": np.ascontiguousarray(h2h_b.reshape(HC, 128).T).astype(np.float32),
        "WgT": np.ascontiguousarray(
            np.concatenate([W_ih[:, :D], W_hh], axis=1).T).astype(BF16),
        "genT": np.ascontiguousarray(gen_w.T).astype(BF16),
        "genbT": np.ascontiguousarray(gen_b.reshape(2, 128).T).astype(np.float32),
        "ident": np.eye(128, dtype=BF16),
        "onesc": np.ones((128, 1), dtype=BF16),
    }
    shared = {"const": const, "Wtok": Wtok}

    nc = build_kernel()
    in_maps = [
        _prep_core(ci, batch_H, text, i2h_w, h2h_w, h2h_b, score_w, W_ih, W_hh,
                   b_ih, b_hh, gen_w, gen_b, shared)
        for ci in range(NC_)
    ]
    import os
    do_trace = bool(int(os.environ.get("KERNEL_TRACE", "0")))
    res = run_bass_kernel_spmd(nc, in_maps, core_ids=list(range(NC_)),
                               trace=do_trace)
    global LAST_RESULT
    LAST_RESULT = res
    outs = res.results  # list of dicts per core
    logits = np.zeros((B, STEPS, V), dtype=np.float32)
    for ci in range(NC_):
        o = outs[ci]["out"] if isinstance(outs[ci], dict) else outs[ci]
        # o [26, 2, 128, 64] -> logits[b, s, vc*128+p]
        logits[ci * BL:(ci + 1) * BL] = o.transpose(3, 0, 1, 2).reshape(BL, STEPS, V)
    return logits


if __name__ == "__main__":
    np.random.seed(0)
    import reference
    inp = {k: np.asarray(v) for k, v in reference.setup_inputs().items()}
    got = kernel(**inp)
    exp = np.asarray(reference.reference(**inp))
    l2 = np.linalg.norm(got - exp) / np.linalg.norm(exp)
    print("l2 rel err:", l2)
